# revision 1
# baseline (speedup 1.0000x reference)
"""Trainium2 Bass kernel for nn_Attention (GQA attention + pairwise bias).

Sharding: 8 cores, sequence-parallel. Core c owns query rows [256c, 256c+256)
and pairwise rows ip in [64c, 64c+64). k/v are computed replicated (1 KV head).
No collectives; host concatenates the 8 output slices.

Layout strategy (transposed-sim attention):
  - all HBM loads on one sync HWDGE FIFO in priority order: rotary, W_qkv
    kv-cols, x, W_qkv q-cols, pairwise, W_out. Compute starts ~10us in.
  - xT via PE transposes; qkv natural; RMSNorm/RoPE on natural tiles
  - q,k transposed (PE) post-norm -> sim computed transposed [j, i]
  - softcap => no max subtraction; softmax denominator via ones-column in v
  - pairwise bias: BN+GELU fused into the transpose-copy (per-partition
    scale/bias on ACT, batched [C,2048]), tiny matmul vs W_bias, expansion
    matmuls accumulate the bias directly into the sim PSUM tiles
  - stage E in 4-head groups: tanh/exp batched [128,1024]; AV accumulated
    transposed into packed per-head PSUM banks; normalize by S; out proj
"""
import numpy as np

N = 2048          # tokens
DIM = 1024
H = 8             # query heads
D_QK = 128
D_V = 192
QKV_COLS = H * D_QK + D_QK + D_V   # 1344
PW = 512          # pairwise i/j
C = 128           # pairwise channels
NCORES = 8
NB = N // NCORES            # 256 own tokens per core
NO = NB // 128              # 2 own token chunks
NT = N // 128               # 16 token chunks
DK = DIM // 128             # 8 dim chunks
IPB = PW // NCORES          # 64 own pairwise rows
PWROWS = IPB * PW           # 32768 flattened own pairwise rows
RT = PWROWS // 128          # 256 pairwise row tiles
SCALE = 64 ** -0.5
SOFTCLAMP = 5.0
RMS_EPS = 1.1920929e-07
BN_EPS = 1e-05
PI_2 = 1.5707963267948966


def build_kernel():
    from concourse import bass, bacc, mybir
    from concourse.tile import TileContext
    from concourse.masks import make_identity

    f32 = mybir.dt.float32
    b16 = mybir.dt.bfloat16
    AF = mybir.ActivationFunctionType
    OP = mybir.AluOpType

    nc = bacc.Bacc()
    dp = lambda name, shape: nc.declare_dram_parameter(name, shape, f32, isOutput=False)
    x_d = dp("x", [N, DIM])
    xo_d = dp("x_own", [NB, DIM])
    rot_d = dp("rotary", [N, D_QK])
    roto_d = dp("rotary_own", [NB, D_QK])
    pw_d = dp("pairwise", [PWROWS, C])
    wqkv_d = dp("W_qkv", [DIM, QKV_COLS])
    qw_d = dp("q_norm_w", [D_QK])
    kw_d = dp("k_norm_w", [D_QK])
    vw_d = dp("v_norm_w", [D_V])
    bng_d = dp("bn_gamma", [C])
    bnb_d = dp("bn_beta", [C])
    bnv_d = dp("bn_running_var", [C])
    wb_d = dp("W_bias", [C, H])
    wout_d = dp("W_out", [H * D_V, DIM])
    out_d = nc.declare_dram_parameter("out", [NB, DIM], f32, isOutput=True)

    with TileContext(nc) as tc:
        import contextlib
        with contextlib.ExitStack() as ctx:
            const = ctx.enter_context(tc.tile_pool(name="const", bufs=1))
            persist = ctx.enter_context(tc.tile_pool(name="persist", bufs=1))
            dpool = ctx.enter_context(tc.tile_pool(name="dpool", bufs=1))
            BMTexp = [dpool.tile([128, H, IPB, 4], b16, tag=f"bmt{j}", name=f"bmt{j}") for j in range(4)]
            import contextlib as _cl
            pwstk = ctx.enter_context(_cl.ExitStack())
            pwpool = pwstk.enter_context(tc.tile_pool(name="pwpool", bufs=6))

            # identity first: the very first PE transposes depend on it
            id128 = const.tile([128, 128], b16)
            make_identity(nc, id128)

            # ---- priority DMAs (sync HWDGE FIFO: strict issue order) ----
            rot_v = rot_d.rearrange("(a p) c -> p a c", p=128)
            rotn = const.tile([128, NT, D_QK], f32)
            nc.sync.dma_start(out=rotn, in_=rot_v[:, :, :])
            roto = const.tile([128, NO, D_QK], f32)
            nc.sync.dma_start(out=roto, in_=roto_d.rearrange("(a p) c -> p a c", p=128)[:, :, :])
            wkv = [const.tile([128, 320], b16, tag=f"wkv{k}", name=f"wkv{k}") for k in range(DK)]
            wq_v = wqkv_d.rearrange("(a p) c -> p a c", p=128)
            for k in range(DK):
                nc.gpsimd.dma_start(out=wkv[k], in_=wq_v[:, k, 1024:1344])

            # ---- constants (gpsimd/vector while sync FIFO streams) ----
            E2 = [const.tile([128, 128], b16, tag=f"E2_{a}", name=f"E2_{a}")
                  for a in range(4)]
            cPI2 = const.tile([128, 1], f32)
            nc.vector.memset(cPI2, PI_2)
            cEPS = const.tile([128, 1], f32)
            nc.vector.memset(cEPS, RMS_EPS)
            wbias = const.tile([C, H], b16)
            nc.gpsimd.dma_start(out=wbias, in_=wb_d[:, :])
            # per-partition bn scale/beta vectors [128,1]
            bng = const.tile([C, 1], f32)
            bnb = const.tile([C, 1], f32)
            bnv = const.tile([C, 1], f32)
            for t, d in ((bng, bng_d), (bnb, bnb_d), (bnv, bnv_d)):
                nc.sync.dma_start(out=t, in_=d.rearrange("(c one) -> c one", one=1))
            bnsc = const.tile([C, 1], f32)
            tmpg = const.tile([C, 1], f32)
            # norm-weight broadcast tiles
            kw_bc = const.tile([128, D_QK], f32)
            qw_bc = const.tile([128, D_QK], f32)
            vw_bc = const.tile([128, D_V], f32)
            for t, d, w in ((kw_bc, kw_d, D_QK), (qw_bc, qw_d, D_QK), (vw_bc, vw_d, D_V)):
                dap = d.ap()
                src = bass.AP(tensor=dap.tensor, offset=dap.offset, ap=[[0, 128], [1, w]])
                nc.sync.dma_start(out=t, in_=src)
            # fold SCALE into q norm weight
            nc.vector.tensor_scalar_mul(out=qw_bc, in0=qw_bc, scalar1=SCALE)
            # swapped-half weight tiles (for rotate_half * sin)
            kw_sw = const.tile([128, D_QK], f32)
            qw_sw = const.tile([128, D_QK], f32)
            for sw, bc in ((kw_sw, kw_bc), (qw_sw, qw_bc)):
                nc.vector.tensor_copy(out=sw[:, 0:64], in_=bc[:, 64:128])
                nc.vector.tensor_copy(out=sw[:, 64:128], in_=bc[:, 0:64])

            # ---- persistent activation buffers ----
            kT = persist.tile([128, N], b16)            # [d, j]
            qT_all = persist.tile([128, H, NB], b16)
            qT = [qT_all[:, h, :] for h in range(H)]
            v_aug = [persist.tile([128, D_V + 1], b16, tag=f"va{t}", name=f"va{t}") for t in range(NT)]

            # ================= stages A-C =================
            with contextlib.ExitStack() as cctx:
                cpool = cctx.enter_context(tc.tile_pool(name="cpool", bufs=1))
                xpool = cctx.enter_context(tc.tile_pool(name="xpool", bufs=3))
                spool = cctx.enter_context(tc.tile_pool(name="spool", bufs=3))
                kvp = cctx.enter_context(tc.tile_pool(name="kvp", bufs=6))
                qcp = cctx.enter_context(tc.tile_pool(name="qcp", bufs=2))
                vpool = cctx.enter_context(tc.tile_pool(name="vpool", bufs=4))
                pst_p = cctx.enter_context(tc.tile_pool(name="pst", bufs=2, space="PSUM"))
                pkv_p = cctx.enter_context(tc.tile_pool(name="pkv", bufs=2, space="PSUM"))
                pq_p = cctx.enter_context(tc.tile_pool(name="pq", bufs=1, space="PSUM"))

                # load x (bf16); priority: x, xon, wq, then pairwise
                xT_all = cpool.tile([128, DK, N], b16)
                xTo_all = cpool.tile([128, DK, NB], b16)
                xT = [xT_all[:, k, :] for k in range(DK)]
                xTo = [xTo_all[:, k, :] for k in range(DK)]
                x_v = x_d.rearrange("(a p) c -> p a c", p=128)   # [128, 16, 1024]
                xns = []
                for g in range(8):
                    xn = xpool.tile([128, 2, DIM], b16, tag="xn", name="xn")
                    nc.gpsimd.dma_start(out=xn, in_=x_v[:, 2 * g:2 * (g + 1), :])
                    xns.append(xn)
                xo_v = xo_d.rearrange("(a p) c -> p a c", p=128)  # [128, 2, 1024]
                xon = xpool.tile([128, NO, DIM], b16, tag="xon", name="xon")
                nc.gpsimd.dma_start(out=xon, in_=xo_v[:, :, :])
                wq = [cpool.tile([128, 1024], b16, tag=f"wq{k}", name=f"wq{k}") for k in range(DK)]
                for k in range(DK):
                    nc.gpsimd.dma_start(out=wq[k], in_=wq_v[:, k, 0:1024])
                pw_view = pw_d.rearrange("(a p) c -> p a c", p=128)  # [128, 256, 128]
                pw_sb = []
                for ch in range(8):
                    t = pwpool.tile([128, RT // 8, C], b16, tag="pw", name=f"pw{ch}")
                    nc.gpsimd.dma_start(out=t, in_=pw_view[:, 32 * ch:32 * (ch + 1), :])
                    pw_sb.append(t)
                # E2 fills issued after the DMA queue is primed
                for a in range(4):
                    ev = E2[a].rearrange("p (j r) -> p j r", r=4)
                    nc.gpsimd.memset(E2[a], 0.0)
                    nc.gpsimd.affine_select(
                        out=ev, in_=ev, compare_op=OP.not_equal, fill=1.0,
                        base=-32 * a, pattern=[[-1, 32], [0, 4]], channel_multiplier=1)

                # x transposes (chase the DMA chunks)
                for g in range(8):
                    xn = xns[g]
                    for a in range(2):
                        tcn = 2 * g + a
                        ps = pst_p.tile([128, 1024], b16, tag="pst", name="pst")
                        for k in range(DK):
                            nc.tensor.transpose(ps[:, 128 * k:128 * (k + 1)],
                                                xn[:, a, 128 * k:128 * (k + 1)], id128)
                        dst = bass.AP(tensor=xT_all.tensor, offset=xT_all.offset + 128 * tcn,
                                      ap=[xT_all.ap[0], [N, DK], [1, 128]])
                        # alternate copy engine to balance DVE/ACT load
                        if tcn % 2 == 0:
                            nc.vector.tensor_copy(out=dst, in_=ps)
                        else:
                            nc.scalar.copy(out=dst, in_=ps)
                for a in range(NO):
                    ps = pst_p.tile([128, 1024], b16, tag="pst", name="pst")
                    for k in range(DK):
                        nc.tensor.transpose(ps[:, 128 * k:128 * (k + 1)],
                                            xon[:, a, 128 * k:128 * (k + 1)], id128)
                    dst = bass.AP(tensor=xTo_all.tensor, offset=xTo_all.offset + 128 * a,
                                  ap=[xTo_all.ap[0], [NB, DK], [1, 128]])
                    nc.scalar.copy(out=dst, in_=ps)

                # rotary -> weighted cos/sin tiles; Sin batched over all chunks
                # (sin computed first, then cos written in-place into rotn)
                sn_all = cpool.tile([128, NT, D_QK], f32)
                nc.scalar.activation(out=sn_all, in_=rotn, func=AF.Sin)
                cs_all = rotn
                nc.scalar.activation(out=cs_all, in_=rotn, func=AF.Sin, bias=cPI2)
                sno = cpool.tile([128, NO, D_QK], f32)
                nc.scalar.activation(out=sno, in_=roto, func=AF.Sin)
                cso = roto
                nc.scalar.activation(out=cso, in_=roto, func=AF.Sin, bias=cPI2)
                # bnscale = (gamma+1)*sqrt(C)/sqrt(max(var, BN_EPS))
                nc.vector.tensor_scalar_max(out=bnsc, in0=bnv, scalar1=BN_EPS)
                nc.scalar.activation(out=tmpg, in_=bnsc, func=AF.Sqrt)
                nc.vector.reciprocal(bnsc, tmpg)
                nc.vector.tensor_scalar_add(out=tmpg, in0=bng, scalar1=1.0)
                nc.vector.tensor_mul(bnsc, bnsc, tmpg)
                nc.vector.tensor_scalar_mul(out=bnsc, in0=bnsc, scalar1=float(np.sqrt(C)))
                wcos_k, wsin_k = [], []
                for t in range(NT):
                    nc.vector.tensor_mul(cs_all[:, t, :], cs_all[:, t, :], kw_bc)
                    nc.vector.tensor_mul(sn_all[:, t, :], sn_all[:, t, :], kw_sw)
                    wcos_k.append(cs_all[:, t, :])
                    wsin_k.append(sn_all[:, t, :])
                wcos_q, wsin_q = [], []
                for t in range(NO):
                    nc.vector.tensor_mul(cso[:, t, :], cso[:, t, :], qw_bc)
                    nc.vector.tensor_mul(sno[:, t, :], sno[:, t, :], qw_sw)
                    wcos_q.append(cso[:, t, :])
                    wsin_q.append(sno[:, t, :])

                def rope_only(src_ap, d, rs, wcos, wsin, ps_dst):
                    """RoPE on a pre-normalized src (rs = per-token 1/rms),
                    then PE-transpose into the given psum slice."""
                    hd = d // 2
                    m1 = spool.tile([128, d], f32, tag="m1", name="m1")
                    nc.vector.scalar_tensor_tensor(out=m1, in0=src_ap, scalar=rs,
                                                   in1=wcos, op0=OP.mult, op1=OP.mult)
                    t2 = spool.tile([128, hd], f32, tag="t2", name="t2")
                    rb = spool.tile([128, d], b16, tag="rb", name="rb")
                    nc.vector.scalar_tensor_tensor(out=t2, in0=src_ap[:, hd:d], scalar=rs,
                                                   in1=wsin[:, 0:hd], op0=OP.mult, op1=OP.mult)
                    nc.vector.tensor_sub(rb[:, 0:hd], m1[:, 0:hd], t2)
                    t3 = spool.tile([128, hd], f32, tag="t3", name="t3")
                    nc.vector.scalar_tensor_tensor(out=t3, in0=src_ap[:, 0:hd], scalar=rs,
                                                   in1=wsin[:, hd:d], op0=OP.mult, op1=OP.mult)
                    nc.vector.tensor_add(rb[:, hd:d], m1[:, hd:d], t3)
                    nc.tensor.transpose(ps_dst, rb, id128)

                # k/v for all chunks; PSUM slots recycle fast via an SBUF
                # copy, and sqrt/recip are batched per group of 4 chunks
                kvc = [None] * NT
                if True:
                    for t in range(NT):
                        ps_kv = pkv_p.tile([128, 320], f32, tag="pskv", name="pskv")
                        for k in range(DK):
                            nc.tensor.matmul(ps_kv, xT[k][:, 128 * t:128 * (t + 1)],
                                             wkv[k],
                                             start=(k == 0), stop=(k == DK - 1))
                        kvc[t] = kvp.tile([128, 320], f32, tag="kvc", name=f"kvc{t}")
                        nc.vector.tensor_copy(out=kvc[t], in_=ps_kv)
                        if t % 4 == 0:
                            ssk4 = vpool.tile([128, 4], f32, tag="ssk4", name="ssk4")
                            ssv4 = vpool.tile([128, 4], f32, tag="ssv4", name="ssv4")
                        i4 = t % 4
                        sq = spool.tile([128, D_QK], b16, tag="sq", name="sq")
                        nc.scalar.activation(out=sq, in_=kvc[t][:, 0:D_QK], func=AF.Square,
                                             accum_out=ssk4[:, i4:i4 + 1])
                        sqv = spool.tile([128, D_V], b16, tag="sqv", name="sqv")
                        nc.scalar.activation(out=sqv, in_=kvc[t][:, D_QK:320], func=AF.Square,
                                             accum_out=ssv4[:, i4:i4 + 1])
                        if t % 4 == 3:
                            sdk4 = vpool.tile([128, 4], f32, tag="sdk4", name="sdk4")
                            nc.scalar.activation(out=sdk4, in_=ssk4, func=AF.Sqrt,
                                                 scale=1.0 / D_QK, bias=cEPS)
                            rsk4 = vpool.tile([128, 4], f32, tag="rsk4", name="rsk4")
                            nc.vector.reciprocal(rsk4, sdk4)
                            sdv4 = vpool.tile([128, 4], f32, tag="sdv4", name="sdv4")
                            nc.scalar.activation(out=sdv4, in_=ssv4, func=AF.Sqrt,
                                                 scale=1.0 / D_V, bias=cEPS)
                            rsv4 = vpool.tile([128, 4], f32, tag="rsv4", name="rsv4")
                            nc.vector.reciprocal(rsv4, sdv4)
                            ps_k4 = pst_p.tile([128, 512], b16, tag="pstk", name="pstk")
                            for u in range(4):
                                tt = t - 3 + u
                                rope_only(kvc[tt][:, 0:D_QK], D_QK, rsk4[:, u:u + 1],
                                          wcos_k[tt], wsin_k[tt],
                                          ps_k4[:, 128 * u:128 * (u + 1)])
                                nc.vector.scalar_tensor_tensor(
                                    out=v_aug[tt][:, 0:D_V], in0=kvc[tt][:, D_QK:320],
                                    scalar=rsv4[:, u:u + 1], in1=vw_bc,
                                    op0=OP.mult, op1=OP.mult)
                                nc.vector.memset(v_aug[tt][:, D_V:D_V + 1], 1.0)
                            nc.vector.tensor_copy(out=kT[:, 512 * (t // 4):512 * (t // 4 + 1)],
                                                  in_=ps_k4)

                # q for own chunks (SCALE folded into qw_bc/qw_sw)
                if True:
                    for oc in range(NO):
                        psq0 = pq_p.tile([128, 512], f32, tag="psq0", name="psq0")
                        psq1 = pq_p.tile([128, 512], f32, tag="psq1", name="psq1")
                        for k in range(DK):
                            nc.tensor.matmul(psq0, xTo[k][:, 128 * oc:128 * (oc + 1)],
                                             wq[k][:, 0:512], start=(k == 0), stop=(k == DK - 1))
                            nc.tensor.matmul(psq1, xTo[k][:, 128 * oc:128 * (oc + 1)],
                                             wq[k][:, 512:1024], start=(k == 0), stop=(k == DK - 1))
                        qc = qcp.tile([128, 1024], f32, tag="qc", name=f"qc{oc}")
                        nc.vector.tensor_copy(out=qc[:, 0:512], in_=psq0)
                        nc.vector.tensor_copy(out=qc[:, 512:1024], in_=psq1)
                        ssq8 = vpool.tile([128, H], f32, tag="ssq8", name="ssq8")
                        for h in range(H):
                            sq = spool.tile([128, D_QK], b16, tag="sq", name="sq")
                            nc.scalar.activation(out=sq, in_=qc[:, 128 * h:128 * (h + 1)],
                                                 func=AF.Square, accum_out=ssq8[:, h:h + 1])
                        uq8 = vpool.tile([128, H], f32, tag="uq8", name="uq8")
                        nc.scalar.activation(out=uq8, in_=ssq8, func=AF.Sqrt,
                                             scale=1.0 / D_QK, bias=cEPS)
                        rsq8 = vpool.tile([128, H], f32, tag="rsq8", name="rsq8")
                        nc.vector.reciprocal(rsq8, uq8)
                        ps_q8 = pst_p.tile([128, 1024], b16, tag="pst", name="ps_q8")
                        for h in range(H):
                            rope_only(qc[:, 128 * h:128 * (h + 1)], D_QK, rsq8[:, h:h + 1],
                                      wcos_q[oc], wsin_q[oc],
                                      ps_q8[:, 128 * h:128 * (h + 1)])
                        dstq = bass.AP(tensor=qT_all.tensor, offset=qT_all.offset + 128 * oc,
                                       ap=[qT_all.ap[0], [NB, H], [1, 128]])
                        nc.vector.tensor_copy(out=dstq, in_=ps_q8)

            # ---- stage D: pairwise bias ----
            with tc.tile_pool(name="pd", bufs=2, space="PSUM") as pd_p, \
                 tc.tile_pool(name="pbias", bufs=1, space="PSUM") as pb_p, \
                 tc.tile_pool(name="pgp", bufs=3) as pg_p:
                bias_ps = [pb_p.tile([128, IPB * H], f32, tag=f"bps{j}", name=f"bps{j}")
                           for j in range(4)]
                for g in range(RT // 16):
                    ps = pd_p.tile([128, 2048], b16, tag="psd", name="psd")
                    for j in range(16):
                        rt = 16 * g + j
                        ch, loc = rt // 32, rt % 32
                        nc.tensor.transpose(ps[:, 128 * j:128 * (j + 1)],
                                            pw_sb[ch][:, loc, :], id128)
                    pg = pg_p.tile([C, 2048], b16, tag="pg", name="pg")
                    nc.scalar.activation(out=pg, in_=ps, func=AF.Gelu, scale=bnsc, bias=bnb)
                    for j in range(16):
                        rt = 16 * g + j
                        ip, jpb = rt // 4, rt % 4
                        nc.tensor.matmul(bias_ps[jpb][:, H * ip:H * (ip + 1)],
                                         pg[:, 128 * j:128 * (j + 1)], wbias,
                                         start=True, stop=True)
                # BMTexp layout [jp(128), h(8), ip(64), r(4)] so the E2
                # expansion rhs is a dense [128, 512] block per head-pair
                for jpb in range(4):
                    for r in range(4):
                        dst = bass.AP(tensor=BMTexp[jpb].tensor, offset=BMTexp[jpb].offset + r,
                                      ap=[BMTexp[jpb].ap[0], [4, IPB], [4 * IPB, H]])
                        nc.vector.tensor_copy(out=dst, in_=bias_ps[jpb])
            pwstk.close()  # free the 64KB/partition pairwise buffers

            # ================= stage E: attention =================
            with contextlib.ExitStack() as ectx:
                epool = ectx.enter_context(tc.tile_pool(name="epool", bufs=1))
                expp = ectx.enter_context(tc.tile_pool(name="expp", bufs=12))
                tanp = ectx.enter_context(tc.tile_pool(name="tanp", bufs=3))
                nrmp = ectx.enter_context(tc.tile_pool(name="nrmp", bufs=2))

                woutA = [epool.tile([128, DIM], b16, tag=f"wA{h}", name=f"wA{h}") for h in range(H)]
                woutB = [epool.tile([64, DIM], b16, tag=f"wB{h}", name=f"wB{h}") for h in range(H)]
                for h in range(H):
                    nc.gpsimd.dma_start(out=woutA[h], in_=wout_d[192 * h:192 * h + 128, :])
                    nc.gpsimd.dma_start(out=woutB[h], in_=wout_d[192 * h + 128:192 * (h + 1), :])
                oT0 = [epool.tile([128, NB], b16, tag=f"oT0{h}", name=f"oT0{h}") for h in range(H)]
                oT1 = [epool.tile([64, NB], b16, tag=f"oT1{h}", name=f"oT1{h}") for h in range(H)]

                estk = ectx.enter_context(contextlib.ExitStack())
                psim = estk.enter_context(tc.tile_pool(name="psim", bufs=3, space="PSUM"))
                pav = estk.enter_context(tc.tile_pool(name="pav", bufs=1, space="PSUM"))
                # flattened pipeline over units i = 8*pair + jp2:
                # sim-unit SU[i] = 8 matmuls + tanh + exp for 2 jc x 2 heads;
                # AV-unit AU[i] = 8 AV matmuls for those jc. AU lags SU by
                # LAG units so the PE never blocks on the ACT tanh/exp.
                LAG = 3
                NP2 = NT // 2
                po_all, exp_all = {}, {}

                def emit_su(i):
                    pair, jp2 = divmod(i, NP2)
                    h0 = 2 * pair
                    if jp2 == 0:
                        # one single-bank accumulator per (pair, dv-half);
                        # each holds both heads side by side in ONE
                        # accumulation group (bank-safe)
                        po_all[pair] = {
                            dvh: pav.tile([128, 2 * NB], f32, tag=f"po{dvh}", name=f"po{dvh}")
                            for dvh in range(2)}
                    sims = psim.tile([128, 4 * NB], f32, tag="sim", name="sim")
                    for q2 in range(2):
                        jc = 2 * jp2 + q2
                        a, jpb = jc % 4, jc // 4
                        sv = sims[:, 512 * q2:512 * (q2 + 1)]
                        qpair = qT_all[:, h0:h0 + 2, :]
                        nc.tensor.matmul(sv, kT[:, 128 * jc:128 * (jc + 1)], qpair,
                                         start=True, stop=False)
                        rhs = bass.AP(tensor=BMTexp[jpb].tensor,
                                      offset=BMTexp[jpb].offset + NB * h0,
                                      ap=[BMTexp[jpb].ap[0], [1, 512]])
                        nc.tensor.matmul(sv, E2[a], rhs,
                                         start=False, stop=True,
                                         skip_group_check=True)
                    tn = tanp.tile([128, 4 * NB], b16, tag="tanh", name="tanh")
                    nc.scalar.activation(out=tn, in_=sims, func=AF.Tanh,
                                         scale=1.0 / SOFTCLAMP)
                    ex = expp.tile([128, 4 * NB], b16, tag="expT", name="expT")
                    nc.scalar.activation(out=ex, in_=tn, func=AF.Exp, scale=SOFTCLAMP)
                    exp_all[i] = ex

                def emit_au(i):
                    pair, jp2 = divmod(i, NP2)
                    hs = (2 * pair, 2 * pair + 1)
                    po = po_all[pair]
                    ex = exp_all.pop(i)
                    for q2 in range(2):
                        jc = 2 * jp2 + q2
                        for dvh in range(2):
                            lhs = v_aug[jc][:, 0:128] if dvh == 0 else v_aug[jc][:, 128:193]
                            rows = 128 if dvh == 0 else 65
                            nc.tensor.matmul(
                                po[dvh][0:rows, :], lhs,
                                ex[:, 512 * q2:512 * (q2 + 1)],
                                start=(jc == 0), stop=(jc == NT - 1))
                    if jp2 == NP2 - 1:
                        for u, h in enumerate(hs):
                            s_sb = nrmp.tile([1, NB], f32, tag="ssb", name="ssb")
                            nc.vector.tensor_copy(out=s_sb, in_=po[1][64:65, NB * u:NB * (u + 1)])
                            rs = nrmp.tile([1, NB], f32, tag="rsb", name="rsb")
                            nc.vector.reciprocal(rs, s_sb)
                            rsb = nrmp.tile([128, NB], f32, tag="rsbc", name="rsbc")
                            nc.gpsimd.partition_broadcast(rsb, rs)
                            nc.vector.tensor_mul(oT0[h], po[0][:, NB * u:NB * (u + 1)], rsb)
                            nc.vector.tensor_mul(oT1[h], po[1][0:64, NB * u:NB * (u + 1)], rsb[0:64, :])

                NU = (H // 2) * NP2
                for i in range(NU + LAG):
                    if i < NU:
                        emit_su(i)
                    if i >= LAG:
                        emit_au(i - LAG)

                # ================= stage F: output projection =================
                estk.close()  # free attention PSUM pools before stage F
                with tc.tile_pool(name="pf", bufs=2, space="PSUM") as pf_p, \
                     tc.tile_pool(name="fo", bufs=2) as fo_p:
                    for ic in range(NO):
                        for nh in range(2):
                            pf = pf_p.tile([128, 512], f32, tag="pf", name="pf")
                            for h in range(H):
                                nc.tensor.matmul(pf, oT0[h][:, 128 * ic:128 * (ic + 1)],
                                                 woutA[h][:, 512 * nh:512 * (nh + 1)],
                                                 start=(h == 0), stop=False)
                                nc.tensor.matmul(pf, oT1[h][:, 128 * ic:128 * (ic + 1)],
                                                 woutB[h][:, 512 * nh:512 * (nh + 1)],
                                                 start=False, stop=(h == H - 1))
                            osb = fo_p.tile([128, 512], f32, tag="osb", name="osb")
                            nc.vector.tensor_copy(out=osb, in_=pf)
                            nc.sync.dma_start(
                                out=out_d[128 * ic:128 * (ic + 1), 512 * nh:512 * (nh + 1)],
                                in_=osb)
    nc.compile()
    return nc


_NC = None


def kernel(x, pairwise, rotary_emb, W_qkv, q_norm_w, k_norm_w, v_norm_w,
           bn_gamma, bn_beta, bn_running_var, W_bias, W_out):
    global _NC
    from concourse.bass_utils import run_bass_kernel_spmd
    if _NC is None:
        _NC = build_kernel()
    f = lambda a: np.ascontiguousarray(np.asarray(a), dtype=np.float32)
    xf = f(x)[0]
    pwf = f(pairwise)[0].reshape(PW * PW, C)
    rotf = f(rotary_emb)
    base = {
        "x": xf, "rotary": rotf, "W_qkv": f(W_qkv),
        "q_norm_w": f(q_norm_w), "k_norm_w": f(k_norm_w), "v_norm_w": f(v_norm_w),
        "bn_gamma": f(bn_gamma), "bn_beta": f(bn_beta),
        "bn_running_var": f(bn_running_var), "W_bias": f(W_bias), "W_out": f(W_out),
    }
    in_maps = []
    for c in range(NCORES):
        m = dict(base)
        m["x_own"] = xf[NB * c:NB * (c + 1)]
        m["rotary_own"] = rotf[NB * c:NB * (c + 1)]
        m["pairwise"] = pwf[PWROWS * c:PWROWS * (c + 1)]
        in_maps.append(m)
    res = run_bass_kernel_spmd(_NC, in_maps, list(range(NCORES)))
    out = np.concatenate([res.results[c]["out"] for c in range(NCORES)], axis=0)
    return out[None].astype(np.float32)

